# revision 1
# baseline (speedup 1.0000x reference)
"""Two-layer SAGEConv GNN node classifier on 8 Trainium2 NeuronCores.

Strategy (sharding_hint: shard nodes across cores, weights replicated):
  - Nodes sharded by dst across 8 cores (12500 each, padded to 12544 = 98
    tiles of 128). Each core aggregates messages for its own nodes.
  - Aggregation = selection-matrix matmul: gather x[src] rows (dma_gather,
    int16 indices -> 4 src-range buckets of <=32768 rows), build one-hot
    selection S[e, d] = (rel_dst[e] == d) on DVE, accumulate
    aggT = sum_chunks Gs_c^T.. via PE matmuls into PSUM.
  - h = relu(agg @ W_l + x @ W_r + b): two accumulating matmuls with
    feat-major lhsT ([aggT ; xT_aug]) where xT_aug carries a ones-row for
    the bias.
  - Between layers: AllGather of bf16 h1 slices + SWDGE cast-DMA into a
    replicated f32 gather table per core.
"""

import numpy as np
import ml_dtypes

import concourse.bacc as bacc
import concourse.mybir as mybir
import concourse.tile as tile
from concourse.bass_utils import run_bass_kernel_spmd

# ---- walrus-compat patch: only ONE sync-wait command per instruction ------
from concourse.vector_clock import ScopedClock


def _patched_drain_and_barrier(self, tick_clock, wait_clock):
    probe = self.nc.sync.nop(nofuse=True, hint="drain_wait_probe")
    wait_clock.add_sem_waits(probe.ins, ScopedClock({None: tick_clock.global_clock}))
    si = probe.ins.sync_info
    waits = list(si.on_wait) if si is not None else []
    if len(waits) > 1:
        si.on_wait = waits[:1]
        for i, w in enumerate(waits[1:]):
            n = self.nc.sync.nop(nofuse=True, hint=f"drain_wait_{i}")
            nsi = n.ins.sync_info
            if nsi is None:
                nsi = mybir.SyncInfo(on_wait=[], on_update=[])
                n.ins.sync_info = nsi
            nsi.on_wait = [w]
    self.nc.sync.drain()
    self.nc.all_engine_barrier()
    popped = self.nc._tile_sem_poison_stack.pop()
    assert popped is self._sem_poison
    self.nc.clear_and_free_semaphores(list(self.sems.allocated().values()))


tile.TileContext._drain_and_barrier = _patched_drain_and_barrier
# ---------------------------------------------------------------------------

CORES = 8
N = 100000
C = 64           # feature width (IN_C == HID == 64)
TILE = 128       # dst nodes per output tile
NPC = N // CORES           # 12500 real nodes per core
NT = (NPC + TILE - 1) // TILE   # 98 tiles
NPC_P = NT * TILE               # 12544 padded
GRP = 2                         # tiles per gather group
NG = NT // GRP                  # 49 groups
BUCK = 32768                    # int16 index range per gather bucket
PAD_REL = 160.0                 # is_equal never matches -> zero row

BF16 = mybir.dt.bfloat16
F32 = mybir.dt.float32
I16 = mybir.dt.int16


def _bucket_bounds(nrows):
    bounds = []
    b = 0
    while b < nrows:
        bounds.append((b, min(b + BUCK, nrows)))
        b += BUCK
    assert len(bounds) == 4
    return bounds


def _layer_meta(core, tile_of, relv, rowv, nrows):
    """Per-layer gather metadata. rowv: table row per edge (sorted by nothing).
    Returns (T[4], idx arrays per bucket [CORES][4] -> [NG,128,n16] int16,
    rel array [CORES, 128, NT*CH] f32)."""
    bounds = _bucket_bounds(nrows)
    bk = np.minimum(rowv // BUCK, 3).astype(np.int64)
    key = ((core * NT + tile_of) * 4 + bk)
    order = np.argsort(key, kind="stable")
    skey = key[order]
    cnt = np.bincount(key, minlength=CORES * NT * 4)
    T = []
    for b in range(4):
        mx = int(cnt.reshape(-1, 4)[:, b].max())
        T.append(max(1, -(-mx // 128)))
    CH = int(sum(T))
    offs = np.concatenate([[0], np.cumsum(T)]).astype(np.int64)

    first = np.zeros(CORES * NT * 4 + 1, dtype=np.int64)
    np.add.at(first, key + 1, 1)
    first = np.cumsum(first)[:-1]
    pos = np.arange(len(order)) - first[skey]

    cs = core[order]
    ts = tile_of[order]
    bs = bk[order]
    ids = (rowv[order] - np.array([bo[0] for bo in bounds])[bs]).astype(np.int16)
    rs = relv[order].astype(np.float32)

    idx_arrays = []
    for b in range(4):
        n_slot = GRP * T[b] * 128
        full = np.zeros((CORES, NG, n_slot), dtype=np.int16)
        m = bs == b
        g = ts[m] // GRP
        tin = ts[m] % GRP
        full[cs[m], g, tin * T[b] * 128 + pos[m]] = ids[m]
        w = full.reshape(CORES, NG, -1, 16).swapaxes(2, 3)      # [CORES,NG,16,n16]
        w = np.ascontiguousarray(np.tile(w, (1, 1, 8, 1)))       # [CORES,NG,128,n16]
        idx_arrays.append(w)

    rel_full = np.full((CORES, NT, CH, 128), PAD_REL, dtype=np.float32)
    ch = offs[bs] + pos // 128
    rel_full[cs, ts, ch, pos % 128] = rs
    rel = np.ascontiguousarray(
        rel_full.transpose(0, 3, 1, 2).reshape(CORES, 128, NT * CH))
    return T, idx_arrays, rel, bounds


def _build_program(T1, T2, b_c_val):
    # 3 SWDGE queues: descriptor generation for the three parts of each
    # gather runs concurrently on separate Q7 contexts (~2.6x gen
    # throughput; 4 queues crashes the device on this firmware).
    nc = bacc.Bacc("TRN2", num_devices=CORES, num_swdge_queues=3)

    xtab = nc.dram_tensor("xtab", [N, C], F32, kind="ExternalInput")
    xT_aug = nc.dram_tensor("xT_aug", [C + 1, NPC_P], BF16, kind="ExternalInput")
    invdeg = nc.dram_tensor("invdeg", [C, NPC_P], F32, kind="ExternalInput")
    iota = nc.dram_tensor("iota", [128, 128], BF16, kind="ExternalInput")
    ident = nc.dram_tensor("ident", [128, 128], BF16, kind="ExternalInput")
    wl1 = nc.dram_tensor("wl1", [C, C], BF16, kind="ExternalInput")
    wr1a = nc.dram_tensor("wr1a", [C + 1, C], BF16, kind="ExternalInput")
    wl2 = nc.dram_tensor("wl2", [C, C], BF16, kind="ExternalInput")
    wr2a = nc.dram_tensor("wr2a", [C + 1, C], BF16, kind="ExternalInput")
    wc = nc.dram_tensor("wc", [C, 1], BF16, kind="ExternalInput")
    rel1_d = nc.dram_tensor("rel1", [128, NT * sum(T1)], F32, kind="ExternalInput")
    rel2_d = nc.dram_tensor("rel2", [128, NT * sum(T2)], F32, kind="ExternalInput")
    idx1_d = [nc.dram_tensor(f"idx1_{b}", [NG, 128, GRP * T1[b] * 8], I16,
                             kind="ExternalInput") for b in range(4)]
    idx2_d = [nc.dram_tensor(f"idx2_{b}", [NG, 128, GRP * T2[b] * 8], I16,
                             kind="ExternalInput") for b in range(4)]
    out_d = nc.dram_tensor("out", [NPC_P, 1], F32, kind="ExternalOutput")

    h1slice = nc.dram_tensor("h1slice", [NPC_P, C], BF16)
    h1full_bf = nc.dram_tensor("h1full_bf", [CORES * NPC_P, C], BF16)
    h1full = nc.dram_tensor("h1full", [CORES * NPC_P, C], F32)

    bounds1 = _bucket_bounds(N)
    bounds2 = _bucket_bounds(CORES * NPC_P)

    with tile.TileContext(nc) as tc:
        with (
            tc.tile_pool(name="res", bufs=1) as rp,
            tc.tile_pool(name="gbuf", bufs=4) as gp,
            tc.tile_pool(name="work", bufs=3) as wp,
            tc.tile_pool(name="sel", bufs=4) as sp,
            tc.tile_pool(name="psA", bufs=2, space="PSUM") as psA,
            tc.tile_pool(name="psB", bufs=2, space="PSUM") as psB,
            tc.tile_pool(name="psC", bufs=2, space="PSUM") as psC,
            tc.tile_pool(name="psD", bufs=2, space="PSUM") as psD,
        ):
            iota_sb = rp.tile([128, 128], BF16)
            nc.sync.dma_start(out=iota_sb[:], in_=iota[:])
            ident_sb = rp.tile([128, 128], BF16)
            nc.sync.dma_start(out=ident_sb[:], in_=ident[:])
            xT_sb = rp.tile([C + 1, NPC_P], BF16)
            nc.sync.dma_start(out=xT_sb[:], in_=xT_aug[:])
            inv_sb = rp.tile([C, NPC_P], F32)
            nc.sync.dma_start(out=inv_sb[:], in_=invdeg[:])
            wl1_sb = rp.tile([C, C], BF16)
            nc.sync.dma_start(out=wl1_sb[:], in_=wl1[:])
            wr1_sb = rp.tile([C + 1, C], BF16)
            nc.sync.dma_start(out=wr1_sb[:], in_=wr1a[:])
            wl2_sb = rp.tile([C, C], BF16)
            nc.sync.dma_start(out=wl2_sb[:], in_=wl2[:])
            wr2_sb = rp.tile([C + 1, C], BF16)
            nc.sync.dma_start(out=wr2_sb[:], in_=wr2a[:])
            wc_sb = rp.tile([C, 1], BF16)
            nc.sync.dma_start(out=wc_sb[:], in_=wc[:])
            rel1_sb = rp.tile([128, NT * sum(T1)], F32)
            nc.sync.dma_start(out=rel1_sb[:], in_=rel1_d[:])
            rel2_sb = rp.tile([128, NT * sum(T2)], F32)
            nc.sync.dma_start(out=rel2_sb[:], in_=rel2_d[:])
            h1T_sb = rp.tile([C + 1, NPC_P], BF16)
            nc.vector.memset(h1T_sb[C:C + 1, :], 1.0)

            def emit_layer(T, tab, bnds, idx_d, rel_sb, xTl_sb, wl_sb, wr_sb,
                           is_last):
                CH = sum(T)
                offs = [0, T[0], T[0] + T[1], T[0] + T[1] + T[2]]
                tab_aps = [tab[lo:hi, :] for (lo, hi) in bnds]
                for g in range(NG):
                    gtiles = []
                    for b in range(4):
                        n_idx = GRP * T[b] * 128
                        it = wp.tile([128, n_idx // 16], I16, tag=f"idx{b}")
                        nc.sync.dma_start(out=it[:], in_=idx_d[b][g])
                        gt = gp.tile([128, GRP * T[b], C], F32, tag=f"g{b}")
                        # rotate whole gathers across the 3 SWDGE queues so
                        # descriptor generation for consecutive gathers
                        # overlaps on separate Q7 contexts
                        nc.gpsimd.dma_gather(
                            gt[:], tab_aps[b], it[:], n_idx, n_idx, C,
                            single_packet=False, queue_num=(g * 4 + b) % 3)
                        gtiles.append(gt)
                    for tin in range(GRP):
                        t = g * GRP + tin
                        agg_ps = psA.tile([C, TILE], F32, tag="agg")
                        k = 0
                        for b in range(4):
                            gs = wp.tile([128, T[b], C], BF16, tag=f"gs{b}")
                            nc.vector.tensor_copy(
                                out=gs[:],
                                in_=gtiles[b][:, tin * T[b]:(tin + 1) * T[b], :])
                            for j in range(T[b]):
                                col = t * CH + offs[b] + j
                                s = sp.tile([128, TILE], BF16, tag="sel")
                                nc.vector.tensor_scalar(
                                    out=s[:], in0=iota_sb[:],
                                    scalar1=rel_sb[:, col:col + 1],
                                    scalar2=None,
                                    op0=mybir.AluOpType.is_equal)
                                nc.tensor.matmul(agg_ps[:], lhsT=gs[:, j, :],
                                                 rhs=s[:], start=(k == 0),
                                                 stop=(k == CH - 1))
                                k += 1
                        aggT_sb = wp.tile([C, TILE], BF16, tag="aggT")
                        nc.vector.tensor_tensor(
                            out=aggT_sb[:], in0=agg_ps[:],
                            in1=inv_sb[:, t * TILE:(t + 1) * TILE],
                            op=mybir.AluOpType.mult)
                        h_ps = psB.tile([TILE, C], F32, tag="h")
                        nc.tensor.matmul(h_ps[:], lhsT=aggT_sb[:], rhs=wl_sb[:],
                                         start=True, stop=False)
                        nc.tensor.matmul(h_ps[:],
                                         lhsT=xTl_sb[:, t * TILE:(t + 1) * TILE],
                                         rhs=wr_sb[:], start=False, stop=True)
                        h_sb = wp.tile([TILE, C], BF16, tag="h_sb")
                        nc.scalar.activation(h_sb[:], h_ps[:],
                                             mybir.ActivationFunctionType.Relu)
                        hT_ps = psC.tile([C, TILE], BF16, tag="hT")
                        nc.tensor.transpose(hT_ps[:], h_sb[:], ident_sb[:])
                        if not is_last:
                            nc.sync.dma_start(
                                out=h1slice[t * TILE:(t + 1) * TILE, :],
                                in_=h_sb[:])
                            nc.scalar.activation(
                                h1T_sb[0:C, t * TILE:(t + 1) * TILE], hT_ps[:],
                                mybir.ActivationFunctionType.Copy)
                        else:
                            h2T_sb = wp.tile([C, TILE], BF16, tag="h2T")
                            nc.scalar.activation(
                                h2T_sb[:], hT_ps[:],
                                mybir.ActivationFunctionType.Copy)
                            o_ps = psD.tile([TILE, 1], F32, tag="o")
                            nc.tensor.matmul(o_ps[:], lhsT=h2T_sb[:],
                                             rhs=wc_sb[:], start=True, stop=True)
                            o_sb = wp.tile([TILE, 1], F32, tag="o_sb")
                            nc.scalar.activation(
                                o_sb[:], o_ps[:],
                                mybir.ActivationFunctionType.Copy,
                                bias=float(b_c_val))
                            nc.sync.dma_start(
                                out=out_d[t * TILE:(t + 1) * TILE, :],
                                in_=o_sb[:])

            emit_layer(T1, xtab, bounds1, idx1_d, rel1_sb, xT_sb, wl1_sb,
                       wr1_sb, False)
            nc.gpsimd.collective_compute(
                "AllGather", mybir.AluOpType.bypass,
                replica_groups=[list(range(CORES))],
                ins=[h1slice.ap().opt()], outs=[h1full_bf.ap().opt()])
            nc.gpsimd.dma_start(out=h1full[:], in_=h1full_bf[:])
            emit_layer(T2, h1full, bounds2, idx2_d, rel2_sb, h1T_sb, wl2_sb,
                       wr2_sb, True)

    nc.compile()
    return nc


def _prep_inputs(x, edge_index, W_l1, b_l1, W_r1, W_l2, b_l2, W_r2, W_c):
    src = np.asarray(edge_index[0], dtype=np.int64)
    dst = np.asarray(edge_index[1], dtype=np.int64)
    x = np.asarray(x, dtype=np.float32)

    core = dst // NPC
    dloc = dst - core * NPC
    tile_of = dloc // TILE
    relv = (dloc % TILE).astype(np.float32)

    deg = np.bincount(dst, minlength=N).astype(np.float64)
    inv = (1.0 / np.maximum(deg, 1.0)).astype(np.float32)

    T1, idx1, rel1, _ = _layer_meta(core, tile_of, relv, src, N)
    src_core = src // NPC
    row2 = src_core * NPC_P + (src - src_core * NPC)
    T2, idx2, rel2, _ = _layer_meta(core, tile_of, relv, row2, CORES * NPC_P)

    inv_pad = np.zeros((CORES, NPC_P), dtype=np.float32)
    inv_pad[:, :NPC] = inv.reshape(CORES, NPC)
    inv_rep = np.ascontiguousarray(
        np.broadcast_to(inv_pad[:, None, :], (CORES, C, NPC_P)))

    xT = np.zeros((CORES, C + 1, NPC_P), dtype=ml_dtypes.bfloat16)
    xr = x.reshape(CORES, NPC, C)
    for c in range(CORES):
        xT[c, :C, :NPC] = xr[c].T.astype(ml_dtypes.bfloat16)
        xT[c, C, :] = 1.0

    bf = ml_dtypes.bfloat16
    shared = {
        "xtab": x,
        "iota": np.ascontiguousarray(
            np.broadcast_to(np.arange(128, dtype=np.float32), (128, 128))
        ).astype(bf),
        "ident": np.eye(128, dtype=np.float32).astype(bf),
        "wl1": np.asarray(W_l1, np.float32).astype(bf),
        "wr1a": np.vstack([np.asarray(W_r1, np.float32),
                           np.asarray(b_l1, np.float32)[None, :]]).astype(bf),
        "wl2": np.asarray(W_l2, np.float32).astype(bf),
        "wr2a": np.vstack([np.asarray(W_r2, np.float32),
                           np.asarray(b_l2, np.float32)[None, :]]).astype(bf),
        "wc": np.asarray(W_c, np.float32).astype(bf),
    }
    in_maps = []
    for c in range(CORES):
        m = dict(shared)
        m["xT_aug"] = np.ascontiguousarray(xT[c])
        m["invdeg"] = inv_rep[c]
        m["rel1"] = rel1[c]
        m["rel2"] = rel2[c]
        for b in range(4):
            m[f"idx1_{b}"] = idx1[b][c]
            m[f"idx2_{b}"] = idx2[b][c]
        in_maps.append(m)
    return T1, T2, in_maps


def kernel(x, edge_index, W_l1, b_l1, W_r1, W_l2, b_l2, W_r2, W_c, b_c):
    T1, T2, in_maps = _prep_inputs(x, edge_index, W_l1, b_l1, W_r1, W_l2,
                                   b_l2, W_r2, W_c)
    nc = _build_program(T1, T2, float(np.asarray(b_c).reshape(-1)[0]))
    res = run_bass_kernel_spmd(nc, in_maps, core_ids=list(range(CORES)))
    out = np.concatenate(
        [res.results[c]["out"][:NPC, 0] for c in range(CORES)])
    return out.astype(np.float32)

